# revision 55
# baseline (speedup 1.0000x reference)
"""GroupSort (k=4) Trainium2 Bass kernel, v7 final (~116us HW verified;
staged v2 baseline was 165-175us).

Abandoned variants (measured): DVE self-gathering segment 0 (4x COPY
then TT) is INTERMITTENTLY racy on HW even with a drain between - 2 of
4 runs returned 1e38 garbage; lane-split partial stores into a sliced
dram rearrange view were sim-correct but wrong on HW. Keep ACT as the
only gather engine and whole-segment stores.

x: (16384, 4096) f32. Sort each contiguous group of 4 along the last dim.
Sharding: batch-parallel across 8 NeuronCores (2048 rows/core), no comms.

Measured HW rules (microbench.py / microbench2.py, v4/v5 traces):
  * DVE TensorTensor = 2x (0.6ns/elem) when operand reads are unit runs /
    run-of-2 / long 2-block views AND writes are sequential-ish blocks;
    single-elem strided reads or alternating-block writes = 1x-0.25x.
    scalar_tensor_tensor never exceeds 1x. DVE COPY = 4x on unit.
  * ACT gather (stride-4 single read -> unit write) ~1.05ns/elem.
  * DMA: 16 engines, ~330-342 GB/s/core effective; 32 MiB/core (bf16
    in+out) is the traffic floor (fp8 fails the 2e-2 rel-err gate).
  * DVE cost/segment: 2.26/3.7/6.03/11.43 us for 1024/2048/4096/8192.

Pipeline per segment [128p, Fs free], Gs = Fs/4 (see v4/v5 docstrings):
  SP    loads segment -> tin arena        (ring allocator, overlap waits)
  ACT   4 lane-gathers tin[(g k)] -> ln arena [e0|e1|e2|e3]
  DVE   6-TT network, all 2x shapes, -> w arena [w0|l0|l1|l2|l3|w1]
          s1 min/max([e0|e2],[e1|e3]) -> s1=[p|q|P|Q] lanes
          s2 min/max([p|P],[q|Q])     -> l0->1, w1->5 / w0->0, l3->4
          s3 min/max(w0, w1)          -> l1->2, l2->3
        (all writes increasing uniform 2-blocks; final lanes in-order
        contiguous at w[off+G : off+5G])
  Pool  issues the store DMA (keeps ACT's gather stream unblocked)
  Host  casts x->bf16 pre-upload (monotone rounding, same rel err as
        v2's bf16-output path) and re-interleaves lanes on unshard.

v6 = v5 + ring-arena slot allocation. v5's fixed FMAX-strided slots made
tiny ramp segments burn whole slots, serializing load(i+NBUF) behind
gathers(i) (~10us of DVE ramp idle). Arenas let the ramp stream.
Variable schedule: small head primes DVE by ~11us; 8192 steady tiles
amortize op overhead; small tail keeps the last compute-gated store off
the critical path.
"""

import numpy as np

B, D, K = 16384, 4096, 4
NCORES = 8
RPC = B // NCORES  # rows per core
N = RPC * D  # flat elements per core
P = 128  # SBUF partitions
PPF = N // P  # free elems per partition per core (65536)
FMAX = 8192
# ramp sums to 2*FMAX so arena wraps align with the 8192 steady tiles
# (misaligned wraps chain loads behind old gathers); 4096 plateau feeds
# the first 8192 gather without a DVE stall; small tail keeps the last
# compute-gated store short.
# (A smoother <=4/3-growth ramp with non-power-of-2 sizes would remove
# the remaining ~2.3us of DVE ramp waits per the measured ACT/DVE rates,
# but non-1024-multiple segments returned NaN on HW - sim-correct - so
# the DMA lowering appears to need power-of-2 per-partition chunks.)
# NOTE: smoother ramps using non-power-of-2 sizes (tried 1360-class and
# 512-multiple 1536/3072/4608) both return 1e38-class garbage on HW
# (sim-correct): segment sizes must be power-of-2 elems per partition.
SEGS = [1024, 1024, 2048, 4096, 4096, 4096] + [8192] * 5 + [4096, 2048, 2048]
assert sum(SEGS) == PPF
NSEG = len(SEGS)
CAP_TIN = 3 * FMAX  # tin arena (elems/partition)
CAP_LN = 3 * FMAX  # lane arena (3 deep: keeps ln-reuse waits clear of
# the paired s_net boundaries below)
# DVE drain/sem boundaries merged across segment pairs: each boundary
# costs ~270-450ns of pipeline flush + waits. Pairs chosen so every
# s_net-gated consumer (ACT ln reuse, Pool store issue, w reuse via
# dma_out) still fits its slack - verified against the arena dep chains.
SKIP_DRAIN = {4, 6, 8, 11}  # no boundary after these
INC2_DRAIN = {5, 7, 9, 12}  # pair-closing boundary (inc s_net by 2)
CAP_W = 36864  # w arena: 6*Gs per segment
ORDERED_LOADS = True  # REQUIRED for correctness with UNEQUAL segment
# sizes: the v2 "FIFO drain" argument for skipping completion-ordering
# waits only holds for equal-size loads (per-engine portions stay in
# lockstep). With variable segments, engine skew can satisfy a
# consumer's dma_in threshold while an earlier load is still in flight
# - matches every intermittent 1e38-class failure observed since
# variable segments were introduced (incl. one on a fully-anchored,
# previously-verified build). Serial issue keeps at most one load in
# flight, making threshold==completion; loads retain ~2x headroom over
# the DVE-paced pipeline, costing only ~2-3us of ramp.

_cache = {}


def _arena(sizes, cap):
    """Ring allocator: returns (offsets, deps); deps[i] = largest j<i whose
    range overlaps segment i's range (-1 if none). Consumption is in
    order, so waiting for j covers all overlapping predecessors."""
    offs, deps, ranges = [], [], []
    pos = 0
    for i, sz in enumerate(sizes):
        assert sz <= cap
        if pos + sz > cap:
            pos = 0
        a, b = pos, pos + sz
        dep = -1
        for j, (x, y) in enumerate(ranges):
            if x < b and y > a:
                dep = j
        offs.append(a)
        deps.append(dep)
        ranges.append((a, b))
        pos = b
    return offs, deps


def _ap(t, offset, dims):
    """Raw AP over SBUF tensor t: partition dim + given [stride, count] dims."""
    from concourse.ap import AP

    base = t[:]
    return AP(base.tensor, offset, [list(base.ap[0])] + [list(d) for d in dims])


def _build():
    import concourse.bass as bass
    import concourse.mybir as mybir

    bf16 = mybir.dt.bfloat16
    mn = mybir.AluOpType.min
    mx = mybir.AluOpType.max

    nc = bass.Bass()
    x = nc.dram_tensor("x", [N], bf16, kind="ExternalInput")
    y = nc.dram_tensor("y", [N], bf16, kind="ExternalOutput")

    # segment i = contiguous flat chunk [P*off_i, P*(off_i+Fs)), viewed
    # [P, Fs]; groups of 4 never straddle partitions (all sizes %1024==0).
    seg_off = []
    o = 0
    for fs in SEGS:
        seg_off.append(o)
        o += fs

    to_tin, dep_tin = _arena(SEGS, CAP_TIN)
    to_ln, dep_ln = _arena(SEGS, CAP_LN)
    to_w, dep_w = _arena([6 * (fs // K) for fs in SEGS], CAP_W)

    with (
        nc.sbuf_tensor([P, CAP_TIN], bf16) as tin,
        nc.sbuf_tensor([P, CAP_LN], bf16) as ln,
        nc.sbuf_tensor([P, FMAX], bf16) as s1,
        nc.sbuf_tensor([P, CAP_W], bf16) as w,
        nc.semaphore("dma_in") as dma_in,
        nc.semaphore("dma_out") as dma_out,
        nc.semaphore("s_act") as s_act,
        nc.semaphore("s_net") as s_net,
        nc.semaphore("dma_fin") as dma_fin,
        nc.Block() as block,
    ):

        def x_seg(i):
            fs = SEGS[i]
            return x[P * seg_off[i] : P * (seg_off[i] + fs)].rearrange(
                "(p f) -> p f", p=P
            )

        def y_seg(i):
            fs = SEGS[i]
            return y[P * seg_off[i] : P * (seg_off[i] + fs)].rearrange(
                "(p f) -> p f", p=P
            )

        @block.sync
        def _(sync):
            for i in range(NSEG):
                if ORDERED_LOADS and i > 0:
                    sync.wait_ge(dma_in, 16 * i)
                if dep_tin[i] >= 0:
                    sync.wait_ge(s_act, dep_tin[i] + 1)
                a = to_tin[i]
                sync.dma_start(tin[:, a : a + SEGS[i]], x_seg(i)).then_inc(
                    dma_in, 16
                )

        @block.scalar
        def _(scalar):
            for i in range(NSEG):
                fs = SEGS[i]
                gs = fs // K
                scalar.wait_ge(dma_in, 16 * (i + 1))
                if dep_ln[i] >= 0:
                    scalar.wait_ge(s_net, dep_ln[i] + 1)
                a = to_tin[i]
                lb = to_ln[i]
                tin4 = tin[:, a : a + fs].rearrange("p (g k) -> p g k", k=K)
                for j in range(K):
                    scalar.copy(
                        ln[:, lb + j * gs : lb + (j + 1) * gs], tin4[:, :, j]
                    )
                scalar.drain().then_inc(s_act, 1)
            # final store from ACT (idle by then; HWDGE completion
            # propagation measured faster than the SWDGE path)
            i = NSEG - 1
            gj = SEGS[i] // K
            scalar.wait_ge(s_net, NSEG)
            scalar.dma_start(
                y_seg(i), w[:, to_w[i] + gj : to_w[i] + 5 * gj]
            ).then_inc(dma_fin, 16)
            # REQUIRED: no exit mechanism covers in-flight store
            # completion (HWDGE or SWDGE) - unanchored final stores
            # intermittently return pre-store garbage
            scalar.wait_ge(dma_fin, 16)

        @block.vector
        def _(vector):
            for i in range(NSEG):
                fs = SEGS[i]
                gs = fs // K
                vector.wait_ge(s_act, i + 1)
                if dep_w[i] >= 0:
                    vector.wait_ge(dma_out, 16 * (dep_w[i] + 1))
                lb = to_ln[i]
                A1 = _ap(ln, lb, [[2 * gs, 2], [1, gs]])  # [e0 | e2]
                B1 = _ap(ln, lb + gs, [[2 * gs, 2], [1, gs]])  # [e1 | e3]
                # s1 = [p | q | P | Q] lanes, each gs
                s1m = _ap(s1, 0, [[gs, 2], [1, gs]])  # p->0, q->1
                s1x = _ap(s1, 2 * gs, [[gs, 2], [1, gs]])  # P->2, Q->3
                vector.tensor_tensor(s1m, A1, B1, mn)
                vector.tensor_tensor(s1x, A1, B1, mx)
                A2 = _ap(s1, 0, [[2 * gs, 2], [1, gs]])  # [p | P]
                B2 = _ap(s1, gs, [[2 * gs, 2], [1, gs]])  # [q | Q]
                off = to_w[i]  # w seg layout [w0|l0|l1|l2|l3|w1]
                dmin = _ap(w, off + gs, [[4 * gs, 2], [1, gs]])  # l0->1, w1->5
                dmax = _ap(w, off, [[4 * gs, 2], [1, gs]])  # w0->0, l3->4
                vector.tensor_tensor(dmin, A2, B2, mn)
                vector.tensor_tensor(dmax, A2, B2, mx)
                w0v = w[:, off : off + gs]
                w1v = w[:, off + 5 * gs : off + 6 * gs]
                vector.tensor_tensor(w[:, off + 2 * gs : off + 3 * gs], w0v, w1v, mn)
                vector.tensor_tensor(w[:, off + 3 * gs : off + 4 * gs], w0v, w1v, mx)
                if i in SKIP_DRAIN:
                    continue
                vector.drain().then_inc(s_net, 2 if i in INC2_DRAIN else 1)

        @block.gpsimd
        def _(gpsimd):
            for j in range(NSEG - 1):
                gj = SEGS[j] // K
                gpsimd.wait_ge(s_net, j + 1)
                gpsimd.dma_start(
                    y_seg(j), w[:, to_w[j] + gj : to_w[j] + 5 * gj]
                ).then_inc(dma_out, 16)
            # REQUIRED: neither exit barrier nor SWDGE ring drain covers
            # in-flight store COMPLETION - without an explicit wait the
            # final segment intermittently returns pre-store garbage
            gpsimd.wait_ge(dma_out, 16 * (NSEG - 1))

    return nc


def _run(x_np, trace=False, trace_kwargs=None):
    import ml_dtypes
    from concourse.bass_utils import run_bass_kernel_spmd

    if "nc" not in _cache:
        _cache["nc"] = _build()
    nc = _cache["nc"]

    xb = np.ascontiguousarray(x_np).astype(ml_dtypes.bfloat16)
    shards = np.split(xb, NCORES, axis=0)
    in_maps = [{"x": s.reshape(-1)} for s in shards]
    res = run_bass_kernel_spmd(
        nc,
        in_maps,
        list(range(NCORES)),
        trace=trace,
        **(trace_kwargs or {}),
    )
    outs = []
    for r in res.results:
        yc = np.asarray(r["y"]).reshape(P * PPF)
        parts = []
        o = 0
        for fs in SEGS:
            seg = yc[P * o : P * (o + fs)].reshape(P, K, fs // K)
            parts.append(seg.transpose(0, 2, 1).reshape(-1))
            o += fs
        core = np.concatenate(parts).astype(np.float32)
        outs.append(core.reshape(RPC, D))
    out = np.concatenate(outs, axis=0)
    return out, res


def kernel(x, k):
    assert int(k) == K, f"kernel hardcodes k={K}, got {k}"
    out, _ = _run(np.asarray(x))
    return out


# revision 59
# speedup vs baseline: 1.2424x; 1.2424x over previous
"""GroupSort (k=4) Trainium2 Bass kernel, v7 final (~116us HW verified;
staged v2 baseline was 165-175us).

Abandoned variants (measured): DVE self-gathering segment 0 (4x COPY
then TT) is INTERMITTENTLY racy on HW even with a drain between - 2 of
4 runs returned 1e38 garbage; lane-split partial stores into a sliced
dram rearrange view were sim-correct but wrong on HW. Keep ACT as the
only gather engine and whole-segment stores.

x: (16384, 4096) f32. Sort each contiguous group of 4 along the last dim.
Sharding: batch-parallel across 8 NeuronCores (2048 rows/core), no comms.

Measured HW rules (microbench.py / microbench2.py, v4/v5 traces):
  * DVE TensorTensor = 2x (0.6ns/elem) when operand reads are unit runs /
    run-of-2 / long 2-block views AND writes are sequential-ish blocks;
    single-elem strided reads or alternating-block writes = 1x-0.25x.
    scalar_tensor_tensor never exceeds 1x. DVE COPY = 4x on unit.
  * ACT gather (stride-4 single read -> unit write) ~1.05ns/elem.
  * DMA: 16 engines, ~330-342 GB/s/core effective; 32 MiB/core (bf16
    in+out) is the traffic floor (fp8 fails the 2e-2 rel-err gate).
  * DVE cost/segment: 2.26/3.7/6.03/11.43 us for 1024/2048/4096/8192.

Pipeline per segment [128p, Fs free], Gs = Fs/4 (see v4/v5 docstrings):
  SP    loads segment -> tin arena        (ring allocator, overlap waits)
  ACT   4 lane-gathers tin[(g k)] -> ln arena [e0|e1|e2|e3]
  DVE   6-TT network, all 2x shapes, -> w arena [w0|l0|l1|l2|l3|w1]
          s1 min/max([e0|e2],[e1|e3]) -> s1=[p|q|P|Q] lanes
          s2 min/max([p|P],[q|Q])     -> l0->1, w1->5 / w0->0, l3->4
          s3 min/max(w0, w1)          -> l1->2, l2->3
        (all writes increasing uniform 2-blocks; final lanes in-order
        contiguous at w[off+G : off+5G])
  Pool  issues the store DMA (keeps ACT's gather stream unblocked)
  Host  casts x->bf16 pre-upload (monotone rounding, same rel err as
        v2's bf16-output path) and re-interleaves lanes on unshard.

v6 = v5 + ring-arena slot allocation. v5's fixed FMAX-strided slots made
tiny ramp segments burn whole slots, serializing load(i+NBUF) behind
gathers(i) (~10us of DVE ramp idle). Arenas let the ramp stream.
Variable schedule: small head primes DVE by ~11us; 8192 steady tiles
amortize op overhead; small tail keeps the last compute-gated store off
the critical path.
"""

import numpy as np

B, D, K = 16384, 4096, 4
NCORES = 8
RPC = B // NCORES  # rows per core
N = RPC * D  # flat elements per core
P = 128  # SBUF partitions
PPF = N // P  # free elems per partition per core (65536)
FMAX = 8192
# ramp sums to 2*FMAX so arena wraps align with the 8192 steady tiles
# (misaligned wraps chain loads behind old gathers); 4096 plateau feeds
# the first 8192 gather without a DVE stall; small tail keeps the last
# compute-gated store short.
# (A smoother <=4/3-growth ramp with non-power-of-2 sizes would remove
# the remaining ~2.3us of DVE ramp waits per the measured ACT/DVE rates,
# but non-1024-multiple segments returned NaN on HW - sim-correct - so
# the DMA lowering appears to need power-of-2 per-partition chunks.)
# UNIFORM segments: the pipelined (no-ordering-wait) load path is only
# race-free when all loads are equal size (per-engine DMA portions stay
# in lockstep, so dma_in thresholds imply completion). Variable-size
# schedules raced intermittently (1e38-class garbage, ~1/3 of runs) and
# fixing them with serialized loads cost ~35us. Uniform 4096 + pipelined
# loads matched ~117us with zero failures across the whole session.
SEGS = [4096] * 16
assert sum(SEGS) == PPF
NSEG = len(SEGS)
CAP_TIN = 3 * FMAX  # tin arena (elems/partition)
CAP_LN = 3 * FMAX  # lane arena (3 deep: keeps ln-reuse waits clear of
# the paired s_net boundaries below)
# DVE drain/sem boundaries merged across segment pairs: each boundary
# costs ~270-450ns of pipeline flush + waits. Pairs chosen so every
# s_net-gated consumer (ACT ln reuse, Pool store issue, w reuse via
# dma_out) still fits its slack - verified against the arena dep chains.
SKIP_DRAIN = set()  # per-segment boundaries: uniform schedule has
INC2_DRAIN = set()  # ample arena slack; pairing gains were sub-noise
CAP_W = 36864  # w arena: 6*Gs per segment
ORDERED_LOADS = False  # pipelined loads are safe ONLY because SEGS is
# uniform (see above); set True for CoreSim or any non-uniform schedule

_cache = {}


def _arena(sizes, cap):
    """Ring allocator: returns (offsets, deps); deps[i] = largest j<i whose
    range overlaps segment i's range (-1 if none). Consumption is in
    order, so waiting for j covers all overlapping predecessors."""
    offs, deps, ranges = [], [], []
    pos = 0
    for i, sz in enumerate(sizes):
        assert sz <= cap
        if pos + sz > cap:
            pos = 0
        a, b = pos, pos + sz
        dep = -1
        for j, (x, y) in enumerate(ranges):
            if x < b and y > a:
                dep = j
        offs.append(a)
        deps.append(dep)
        ranges.append((a, b))
        pos = b
    return offs, deps


def _ap(t, offset, dims):
    """Raw AP over SBUF tensor t: partition dim + given [stride, count] dims."""
    from concourse.ap import AP

    base = t[:]
    return AP(base.tensor, offset, [list(base.ap[0])] + [list(d) for d in dims])


def _build():
    import concourse.bass as bass
    import concourse.mybir as mybir

    bf16 = mybir.dt.bfloat16
    mn = mybir.AluOpType.min
    mx = mybir.AluOpType.max

    nc = bass.Bass()
    x = nc.dram_tensor("x", [N], bf16, kind="ExternalInput")
    y = nc.dram_tensor("y", [N], bf16, kind="ExternalOutput")

    # segment i = contiguous flat chunk [P*off_i, P*(off_i+Fs)), viewed
    # [P, Fs]; groups of 4 never straddle partitions (all sizes %1024==0).
    seg_off = []
    o = 0
    for fs in SEGS:
        seg_off.append(o)
        o += fs

    to_tin, dep_tin = _arena(SEGS, CAP_TIN)
    to_ln, dep_ln = _arena(SEGS, CAP_LN)
    to_w, dep_w = _arena([6 * (fs // K) for fs in SEGS], CAP_W)

    with (
        nc.sbuf_tensor([P, CAP_TIN], bf16) as tin,
        nc.sbuf_tensor([P, CAP_LN], bf16) as ln,
        nc.sbuf_tensor([P, FMAX], bf16) as s1,
        nc.sbuf_tensor([P, CAP_W], bf16) as w,
        nc.semaphore("dma_in") as dma_in,
        nc.semaphore("dma_out") as dma_out,
        nc.semaphore("s_act") as s_act,
        nc.semaphore("s_net") as s_net,
        nc.Block() as block,
    ):

        def x_seg(i):
            fs = SEGS[i]
            return x[P * seg_off[i] : P * (seg_off[i] + fs)].rearrange(
                "(p f) -> p f", p=P
            )

        def y_seg(i):
            fs = SEGS[i]
            return y[P * seg_off[i] : P * (seg_off[i] + fs)].rearrange(
                "(p f) -> p f", p=P
            )

        @block.sync
        def _(sync):
            for i in range(NSEG):
                if ORDERED_LOADS and i > 0:
                    sync.wait_ge(dma_in, 16 * i)
                if dep_tin[i] >= 0:
                    sync.wait_ge(s_act, dep_tin[i] + 1)
                a = to_tin[i]
                sync.dma_start(tin[:, a : a + SEGS[i]], x_seg(i)).then_inc(
                    dma_in, 16
                )

        @block.scalar
        def _(scalar):
            for i in range(NSEG):
                fs = SEGS[i]
                gs = fs // K
                scalar.wait_ge(dma_in, 16 * (i + 1))
                if dep_ln[i] >= 0:
                    scalar.wait_ge(s_net, dep_ln[i] + 1)
                a = to_tin[i]
                lb = to_ln[i]
                tin4 = tin[:, a : a + fs].rearrange("p (g k) -> p g k", k=K)
                for j in range(K):
                    scalar.copy(
                        ln[:, lb + j * gs : lb + (j + 1) * gs], tin4[:, :, j]
                    )
                scalar.drain().then_inc(s_act, 1)
                # v4-proven store path: ALL stores on the ACT/HWDGE queue,
                # one segment behind the gathers. (gpsimd/SWDGE stores are
                # the common element of every intermittently-corrupt build
                # since v5 - suspect SWDGE sem incs do not order against
                # its reads of w, defeating DVE's slot-reuse waits.)
                j = i - 1
                if j >= 0:
                    gj = SEGS[j] // K
                    scalar.wait_ge(s_net, j + 1)
                    scalar.dma_start(
                        y_seg(j), w[:, to_w[j] + gj : to_w[j] + 5 * gj]
                    ).then_inc(dma_out, 16)
            i = NSEG - 1
            gj = SEGS[i] // K
            scalar.wait_ge(s_net, NSEG)
            scalar.dma_start(
                y_seg(i), w[:, to_w[i] + gj : to_w[i] + 5 * gj]
            ).then_inc(dma_out, 16)
            # REQUIRED: no exit mechanism covers in-flight store
            # completion - unanchored final stores return garbage
            scalar.wait_ge(dma_out, 16 * NSEG)

        @block.vector
        def _(vector):
            for i in range(NSEG):
                fs = SEGS[i]
                gs = fs // K
                vector.wait_ge(s_act, i + 1)
                if dep_w[i] >= 0:
                    vector.wait_ge(dma_out, 16 * (dep_w[i] + 1))
                lb = to_ln[i]
                A1 = _ap(ln, lb, [[2 * gs, 2], [1, gs]])  # [e0 | e2]
                B1 = _ap(ln, lb + gs, [[2 * gs, 2], [1, gs]])  # [e1 | e3]
                # s1 = [p | q | P | Q] lanes, each gs
                s1m = _ap(s1, 0, [[gs, 2], [1, gs]])  # p->0, q->1
                s1x = _ap(s1, 2 * gs, [[gs, 2], [1, gs]])  # P->2, Q->3
                vector.tensor_tensor(s1m, A1, B1, mn)
                vector.tensor_tensor(s1x, A1, B1, mx)
                A2 = _ap(s1, 0, [[2 * gs, 2], [1, gs]])  # [p | P]
                B2 = _ap(s1, gs, [[2 * gs, 2], [1, gs]])  # [q | Q]
                off = to_w[i]  # w seg layout [w0|l0|l1|l2|l3|w1]
                dmin = _ap(w, off + gs, [[4 * gs, 2], [1, gs]])  # l0->1, w1->5
                dmax = _ap(w, off, [[4 * gs, 2], [1, gs]])  # w0->0, l3->4
                vector.tensor_tensor(dmin, A2, B2, mn)
                vector.tensor_tensor(dmax, A2, B2, mx)
                w0v = w[:, off : off + gs]
                w1v = w[:, off + 5 * gs : off + 6 * gs]
                vector.tensor_tensor(w[:, off + 2 * gs : off + 3 * gs], w0v, w1v, mn)
                vector.tensor_tensor(w[:, off + 3 * gs : off + 4 * gs], w0v, w1v, mx)
                if i in SKIP_DRAIN:
                    continue
                vector.drain().then_inc(s_net, 2 if i in INC2_DRAIN else 1)

    return nc


def _run(x_np, trace=False, trace_kwargs=None):
    import ml_dtypes
    from concourse.bass_utils import run_bass_kernel_spmd

    if "nc" not in _cache:
        _cache["nc"] = _build()
    nc = _cache["nc"]

    xb = np.ascontiguousarray(x_np).astype(ml_dtypes.bfloat16)
    shards = np.split(xb, NCORES, axis=0)
    in_maps = [{"x": s.reshape(-1)} for s in shards]
    res = run_bass_kernel_spmd(
        nc,
        in_maps,
        list(range(NCORES)),
        trace=trace,
        **(trace_kwargs or {}),
    )
    outs = []
    for r in res.results:
        yc = np.asarray(r["y"]).reshape(P * PPF)
        parts = []
        o = 0
        for fs in SEGS:
            seg = yc[P * o : P * (o + fs)].reshape(P, K, fs // K)
            parts.append(seg.transpose(0, 2, 1).reshape(-1))
            o += fs
        core = np.concatenate(parts).astype(np.float32)
        outs.append(core.reshape(RPC, D))
    out = np.concatenate(outs, axis=0)
    return out, res


def kernel(x, k):
    assert int(k) == K, f"kernel hardcodes k={K}, got {k}"
    out, _ = _run(np.asarray(x))
    return out
